# revision 72
# baseline (speedup 1.0000x reference)
"""Based (2nd-order Taylor linear attention) Trainium2 kernel, v3.

Problem: nn_Based_56719338111472.
  hidden [1, 512, 768] -> q,k (12 heads, f=16), v (12 heads, d=64)
  phi = 2nd-order taylor feature map (D = 1 + 16 + 256 = 273)
  causal linear attention, output projection Wo.

Key identity: phi(q)·phi(k) = 0.5 + ((q·k)+4)^2/32, so the feature map
collapses to a polynomial on the plain q·k score — exact block-causal
quadratic attention with K=16 score matmuls and a Square activation.

Scale folding (keeps the Square output directly consumable by the
num/den matmuls, no elementwise pass in between):
  sq' = 2*(a*s+b)^2 = Square(sqrt(2)*a*s + sqrt(2)*b)   (ACT, one op)
  stored vh = v/2 (folded into the v psum evacuation scale)
  num = vh.T @ sq' (off-diag) + vh.T @ ((sq'+1)*mask) (diag)
        + 0.5*cumsum colsum(v)  [per-chunk vector, added in final stt]
  den = (0.5ones).T @ sq' (off-diag) + (0.5ones).T @ ((sq'+1)*mask) (diag)
        + 64*j  [per-chunk constant, added before reciprocal]

Sharding: head-parallel, 2 heads per core (heads 2c, 2c+1; heads >= 12
are zero-padded virtual heads). Each core computes a row-parallel
partial of proj_o; the host sums per-core partials and reassembles.

Structure: m-chunk pipeline with interleaved emission
(A0 A1 B0 A2 B1 C0 A3 B2 C1 B3 C2 C3) so the PE always has independent
work while a chunk's ACT/DVE chain runs.
  A(j): q/k proj (fused, k evacuated cross-partition), v proj, scores,
        squares, diag mask
  B(j): num/den accumulation, den bias, reciprocal, y = (num+cs)*rden
  C(j): out projection, psum evac, store
"""

import math

import ml_dtypes
import numpy as np

import concourse.bass as bass
import concourse.tile as tile
from concourse import bacc, mybir
from concourse.bass_utils import run_bass_kernel_spmd

# ---- problem constants (hardcoded; kernel.py must be self-contained) ----
L = 512          # sequence length
E = 768          # d_model
F = 16           # feature dim per head
HD = 64          # head dim (v)
NH = 12          # real heads
C = 128          # chunk size
NCH = L // C     # 4 m-chunks
ECH = E // 128   # 6 e-chunks
NCORES = 8
HPC = 2          # heads per core

R2 = math.sqrt(2.0)
_SQ_SCALE = R2 / math.sqrt(32.0)
_SQ_BIAS = R2 * 4.0 / math.sqrt(32.0)

BF16 = mybir.dt.bfloat16
F32 = mybir.dt.float32

N_WARMUP = 8     # PE warmup matmuls (HAM clock ramp) during DMA-in


def build_kernel():
    """Build and compile the per-core Bass program (identical on all cores)."""
    nc = bacc.Bacc("TRN2", debug=False, enable_asserts=False)

    ht_d = nc.dram_tensor("ht", (128, NCH * ECH * C), BF16, kind="ExternalInput").ap()
    wpk_d = nc.dram_tensor("wpk", (128, 2 * ECH * C + C), BF16, kind="ExternalInput").ap()
    wo_d = nc.dram_tensor("wo", (128, E), BF16, kind="ExternalInput").ap()
    outp_d = nc.dram_tensor("outp", (128, NCH * ECH * C), BF16, kind="ExternalOutput").ap()

    VOFF = ECH * C           # 768: wv offset inside wpk
    MOFF = 2 * ECH * C       # 1536: mask offset inside wpk

    with tile.TileContext(nc) as tc:
        with (
            tc.tile_pool(name="const", bufs=1) as const_pool,
            tc.tile_pool(name="work", bufs=1) as work,
            tc.tile_pool(name="sc_p", bufs=2) as sc_pool,
            tc.tile_pool(name="y_p", bufs=2) as y_pool,
            tc.tile_pool(name="rd_p", bufs=2) as rd_pool,
            tc.tile_pool(name="o_p", bufs=2) as o_pool,
            tc.tile_pool(name="ps_f", bufs=1, space="PSUM") as ps_f,
            tc.tile_pool(name="ps_blk", bufs=6, space="PSUM") as ps_blk,
        ):
            # ---- input loads: pipeline order on the sync HWDGE ring.
            # wqk/ht0 split in halves so the first projection matmuls start
            # on the e=0..2 chunks while the rest is still in flight. ----
            H3 = 3 * C  # 384: half of an e-block
            wpk_sb = const_pool.tile([128, MOFF + C], BF16, name="wpk_sb")
            ht_sb = const_pool.tile([128, NCH, ECH, C], BF16, name="ht_sb")
            ht_r = ht_d.rearrange("p (j f) -> p j f", j=NCH)
            nc.sync.dma_start(wpk_sb[:, 0:H3], wpk_d[:, 0:H3])      # wqk e<3
            nc.sync.dma_start(ht_sb[:, 0, 0:3], ht_r[:, 0, 0:H3])   # ht0 e<3
            nc.sync.dma_start(wpk_sb[:, H3:VOFF], wpk_d[:, H3:VOFF])
            nc.sync.dma_start(ht_sb[:, 0, 3:6], ht_r[:, 0, H3:])
            nc.sync.dma_start(wpk_sb[:, VOFF:], wpk_d[:, VOFF:])    # wv + mask
            nc.sync.dma_start(ht_sb[:, 1], ht_r[:, 1])
            nc.sync.dma_start(ht_sb[:, 2], ht_r[:, 2])
            nc.sync.dma_start(ht_sb[:, 3], ht_r[:, 3])
            # wo is not needed until the first out-projection (~mid-kernel);
            # issuing it last keeps it from racing the critical early loads
            wo_sb = const_pool.tile([128, E], BF16, name="wo_sb")
            nc.sync.dma_start(wo_sb, wo_d)

            # ---- constants ----
            oh05_sb = const_pool.tile([128, HD], BF16, name="oh05_sb")
            nc.vector.memset(oh05_sb, 0.5)
            ones1_sb = const_pool.tile([128, 1], BF16, name="ones1_sb")
            nc.vector.memset(ones1_sb, 1.0)
            sqbias_sb = const_pool.tile([128, 1], F32, name="sqbias_sb")
            nc.vector.memset(sqbias_sb, _SQ_BIAS)
            csx_sb = const_pool.tile([128, NCH], F32, name="csx_sb")
            nc.vector.memset(csx_sb[:, 0:1], 0.0)
            # tiny dummy activation: ACT LUT table load overlaps the DMA phase
            dummy_sb = const_pool.tile([1, 1], F32, name="dummy_sb")
            nc.scalar.activation(
                dummy_sb,
                sqbias_sb[0:1, :],
                mybir.ActivationFunctionType.Square,
                bias=sqbias_sb[0:1, :],
                scale=1.0,
            )

            # ---- PE warmup during the DMA window (HAM clock ramp) ----
            wu_sb = const_pool.tile([128, 384], BF16, name="wu_sb")
            nc.gpsimd.memset(wu_sb, 0.0)
            ps_wu = ps_blk.tile([128, 512], F32, name="ps_wu", tag="blk")
            for _ in range(N_WARMUP):
                nc.tensor.matmul(
                    ps_wu[:, 0:384], wu_sb[:, 0:128], wu_sb,
                    start=True, stop=True, skip_group_check=True,
                )

            # ---- persistent PSUM banks ----
            ps_num = ps_f.tile([128, L], F32, name="ps_num")
            ps_den = ps_f.tile([128, L], F32, name="ps_den")

            # SBUF work tiles
            # qk_sb: rows 0:16 head0, 32:48 head1; slots 0:4 q, 4:8 k chunks
            qk_sb = work.tile([64, 2 * NCH, C], BF16, name="qk_sb")
            v_sb = work.tile([128, NCH, C], BF16, name="v_sb")

            mask_ap = wpk_sb[:, MOFF : MOFF + C]

            def wv_e(e):
                return wpk_sb[:, VOFF + e * C : VOFF + (e + 1) * C]

            sc_tiles = {}
            rden_tiles = {}
            y_tiles = {}

            def pe_filler(n=256):
                # Garbage matmul into ps_num cols 256:512 (cleared later by
                # the chunk-2/3 groups' opening start=True matmuls): keeps the
                # PE busy across early evac-latency bubbles so the HAM
                # clock-gate sees a fully busy window and unthrottles.
                nc.tensor.matmul(
                    ps_num[:, 256 : 256 + n], wu_sb[:, 0:128], wu_sb[:, 0:n],
                    start=True, stop=True, skip_group_check=True,
                )



            def stage_a(j, fill=False):
                """q/k proj, v proj, scores, squares, diag mask for chunk j."""
                jc = slice(C * j, C * (j + 1))
                # fused q+k projection: psum rows 0:64 q, 64:128 k
                ps_qk = ps_blk.tile([128, C], F32, name="ps_qk", tag="blk")
                for e in range(ECH):
                    nc.tensor.matmul(
                        ps_qk,
                        wpk_sb[:, e * C : (e + 1) * C],
                        ht_sb[:, j, e],
                        start=(e == 0), stop=(e == ECH - 1),
                        skip_group_check=True,
                    )
                if fill:
                    pe_filler()
                    pe_filler()
                nc.vector.tensor_copy(qk_sb[:, j], ps_qk[0:64, :])
                # cross-partition evac (HW-verified): k rows 64:128 -> 0:64
                nc.scalar.copy(qk_sb[:, NCH + j], ps_qk[64:128, :])

                # v projection first: PE stays busy during the q/k evac
                # latency (scores depend on the copies, v only on wv)
                ps_v = ps_blk.tile([128, C + 1], F32, name="ps_v", tag="blk")
                for e in range(ECH):
                    nc.tensor.matmul(
                        ps_v[:, 0:C],
                        ht_sb[:, j, e],
                        wv_e(e),
                        start=(e == 0), stop=(e == ECH - 1),
                        skip_group_check=True,
                    )
                nc.scalar.mul(v_sb[:, j], ps_v[:, 0:C], 0.5)
                if fill:
                    pe_filler()
                    pe_filler()
                # running colsum of vh for the 0.5-term: cs = vh_j.T @ 1
                if j < NCH - 1:
                    nc.tensor.matmul(
                        ps_v[:, C : C + 1], v_sb[:, j], ones1_sb,
                        start=True, stop=True, skip_group_check=True,
                    )
                    nc.vector.tensor_add(
                        csx_sb[:, j + 1 : j + 2],
                        csx_sb[:, j : j + 1],
                        ps_v[:, C : C + 1],
                    )

                # scores for all (j' <= j): two heads in row groups 0/1
                ps_s = [
                    ps_blk.tile([128, 512], F32, name=f"ps_s{h}", tag="blk")
                    for h in range(HPC)
                ]
                for jp in range(j + 1):
                    jpc = slice(C * jp, C * (jp + 1))
                    for h in range(HPC):
                        nc.tensor.matmul(
                            ps_s[h][:, jpc],
                            qk_sb[32 * h : 32 * h + F, NCH + jp],
                            qk_sb[32 * h : 32 * h + F, j],
                            start=True, stop=True, skip_group_check=True,
                        )
                # batched square per head -> sc tile; diag gets (sq'+1)*mask
                nj = C * (j + 1)
                sc = [
                    sc_pool.tile([128, 512], BF16, name=f"sc{h}", tag=f"sc{h}")
                    for h in range(HPC)
                ]
                for h in range(HPC):
                    nc.scalar.activation(
                        sc[h][:, 0:nj],
                        ps_s[h][:, 0:nj],
                        mybir.ActivationFunctionType.Square,
                        bias=sqbias_sb[:, :],
                        scale=_SQ_SCALE,
                    )
                    nc.vector.scalar_tensor_tensor(
                        sc[h][:, C * j : nj],
                        sc[h][:, C * j : nj],
                        1.0,
                        mask_ap,
                        op0=mybir.AluOpType.add,
                        op1=mybir.AluOpType.mult,
                    )
                sc_tiles[j] = sc

            def stage_b(j):
                """num/den accumulation + divide for chunk j (den first so the
                reciprocal overlaps the num matmuls). The den bias 64j and the
                0.5-colsum num corrections are folded in as constant matmuls
                opening each accumulation group."""
                jc = slice(C * j, C * (j + 1))
                sc = sc_tiles[j]
                for jp in range(j + 1):
                    jpc = slice(C * jp, C * (jp + 1))
                    st, sp = (jp == 0), (jp == j)
                    nc.tensor.matmul(
                        ps_den[0:64, jc], oh05_sb, sc[0][:, jpc],
                        start=st, stop=sp, skip_group_check=True,
                    )
                    nc.tensor.matmul(
                        ps_den[64:128, jc], oh05_sb, sc[1][:, jpc],
                        start=st, stop=sp, skip_group_check=True,
                    )
                rden = rd_pool.tile([128, C], F32, name="rden", tag="rden")
                if j == 0:
                    nc.vector.reciprocal_approx_fast(rden, ps_den[:, jc])
                else:
                    dbias = rd_pool.tile([128, C], F32, name="dbias", tag="dbias")
                    nc.vector.tensor_scalar_add(dbias, ps_den[:, jc], 64.0 * j)
                    nc.vector.reciprocal_approx_fast(rden, dbias)
                for jp in range(j + 1):
                    jpc = slice(C * jp, C * (jp + 1))
                    st, sp = (jp == 0), (jp == j)
                    nc.tensor.matmul(
                        ps_num[0:64, jc], v_sb[:, jp, 0:64], sc[0][:, jpc],
                        start=st, stop=sp, skip_group_check=True,
                    )
                    nc.tensor.matmul(
                        ps_num[64:128, jc], v_sb[:, jp, 64:128], sc[1][:, jpc],
                        start=st, stop=sp, skip_group_check=True,
                    )
                y_sb = y_pool.tile([128, C], BF16, name="y_sb", tag="y")
                nc.vector.scalar_tensor_tensor(
                    y_sb,
                    ps_num[:, jc],
                    csx_sb[:, j : j + 1],
                    rden,
                    op0=mybir.AluOpType.add,
                    op1=mybir.AluOpType.mult,
                )
                rden_tiles[j] = rden
                y_tiles[j] = y_sb

            def stage_c(j, last=False):
                """out projection + store for chunk j (flat o_sb, split
                evacuation + stores for a short tail)."""
                y_sb = y_tiles[j]
                ps_o1 = ps_blk.tile([128, 512], F32, name="ps_o1", tag="blk")
                ps_o2 = ps_blk.tile([128, 512], F32, name="ps_o2", tag="blk")
                for cc in range(4):
                    nc.tensor.matmul(
                        ps_o1[:, C * cc : C * (cc + 1)],
                        wo_sb[:, C * cc : C * (cc + 1)], y_sb,
                        start=True, stop=True, skip_group_check=True,
                    )
                o_sb = o_pool.tile([128, ECH * C], BF16, name="o_sb", tag="o")
                ob = ECH * C * j
                nc.vector.tensor_copy(o_sb[:, 0:512], ps_o1)
                nc.sync.dma_start(outp_d[:, ob : ob + 512], o_sb[:, 0:512])
                for cc in range(4, ECH):
                    nc.tensor.matmul(
                        ps_o2[:, C * (cc - 4) : C * (cc - 3)],
                        wo_sb[:, C * cc : C * (cc + 1)], y_sb,
                        start=True, stop=True, skip_group_check=True,
                    )
                nc.scalar.copy(o_sb[:, 512:768], ps_o2[:, 0:256])
                # last chunk: second store on the (by then idle) scalar HWDGE
                # ring so the two final stores issue + drain in parallel;
                # mid-kernel the ACT sequencer is busy with Squares, so the
                # other chunks stay on sync.
                eng = nc.scalar if last else nc.sync
                eng.dma_start(outp_d[:, ob + 512 : ob + 768], o_sb[:, 512:768])

            # interleaved software pipeline
            stage_a(0, fill=True)
            stage_a(1, fill=True)
            stage_b(0)
            stage_a(2)
            stage_b(1)
            stage_c(0)
            stage_a(3)
            stage_b(2)
            stage_c(1)
            # C2 before B3: C2's matmuls depend only on y(2) and fill the PE
            # while B3 waits on chunk-3's squares (ACT); otherwise B3's
            # sc-gated matmuls block C2's ready work in the PE FIFO.
            stage_c(2)
            stage_b(3)
            stage_c(3, last=True)

    nc.compile()
    return nc


def make_core_inputs(hidden_states, Wq, Wk, Wv, Wo):
    """Host-side marshalling: transpose/cast/shard the full inputs."""
    bf16 = ml_dtypes.bfloat16

    hT = np.ascontiguousarray(hidden_states[0].T).astype(np.float32)  # [768, 512]
    ht = (
        hT.reshape(ECH, 128, NCH, C)
        .transpose(1, 2, 0, 3)
        .reshape(128, NCH * ECH * C)
        .astype(bf16)
    )
    maskT = np.triu(np.ones((C, C), np.float32))  # keep n <= m

    WqT = Wq.astype(np.float32).T  # [768, 192]
    WkT = Wk.astype(np.float32).T
    WvT = Wv.astype(np.float32).T  # [768, 768]

    def fold(w):  # [768, X] -> [128, ECH * X] chunk-major
        x = w.shape[1]
        return w.reshape(ECH, 128, x).transpose(1, 0, 2).reshape(128, ECH * x)

    in_maps = []
    for c in range(NCORES):
        # wqk cols: [q_h0 @0 | q_h1 @32 | k_h0 @64 | k_h1 @96]
        wqk = np.zeros((E, 128), np.float32)
        wv = np.zeros((E, 128), np.float32)
        wo = np.zeros((128, E), np.float32)
        for hh in range(HPC):
            head = HPC * c + hh
            if head >= NH:
                continue
            b = 32 * hh
            wqk[:, b : b + F] = WqT[:, F * head : F * (head + 1)]
            wqk[:, 64 + b : 64 + b + F] = WkT[:, F * head : F * (head + 1)]
            wv[:, 64 * hh : 64 * hh + HD] = WvT[:, HD * head : HD * (head + 1)]
            wo[64 * hh : 64 * hh + HD, :] = Wo[:, HD * head : HD * (head + 1)].T
        wpk = np.concatenate([fold(wqk), fold(wv), maskT], axis=1)
        in_maps.append(
            {
                "ht": ht,
                "wpk": wpk.astype(bf16),
                "wo": wo.astype(bf16),
            }
        )
    return in_maps


def unshard(results):
    """Sum row-parallel partials and reassemble [1, L, E] fp32."""
    ncores_real = (NH + HPC - 1) // HPC
    acc = np.zeros((128, NCH * ECH * C), np.float64)
    for c in range(ncores_real):
        acc += results[c]["outp"].astype(np.float64)
    outT = (
        acc.reshape(128, NCH, ECH, C)
        .transpose(2, 0, 1, 3)
        .reshape(E, L)
    )
    return outT.T.astype(np.float32).reshape(1, L, E)


_NC_CACHE = {}


def kernel(hidden_states, Wq, Wk, Wv, Wo):
    if "nc" not in _NC_CACHE:
        _NC_CACHE["nc"] = build_kernel()
    nc = _NC_CACHE["nc"]
    in_maps = make_core_inputs(hidden_states, Wq, Wk, Wv, Wo)
    res = run_bass_kernel_spmd(nc, in_maps, core_ids=list(range(NCORES)))
    return unshard(res.results)
